# revision 29
# baseline (speedup 1.0000x reference)
"""Neural CDE (Tsit5 scan over cubic-interp control) on 8 Trainium2 cores.

Strategy: pure data parallelism over batch (64 -> 8 per core), fp16 on-chip
arithmetic (4x less rounding noise than bf16 at identical engine speed).

Key structural points vs the v1 kernel:
- The Tsit5 stage combinations y_j = y + dt*sum(a_jl k_l) are folded into the
  PE as psum accumulation against pre-scaled weight copies Wa[j,l] = a_jl*W0:
  psum1_j = W0 ybf + sum_l Wa[j,l] @ (dt k_l partials). This removes the DVE
  partial-sum chain and the yin materialization from the critical path; all
  but the freshest-partial matmuls are emitted a stage early and execute in
  the shadow of the W2 chunk stream.
- The step update y_{n+1} = y + sum B_l dt k_l likewise becomes the stage-1
  psum accumulation of the next step via Wb[l] = B_l*W0 copies.
- b2 bias enters psum3 as a single rank-32 matmul (b2iT indicator trick).
- tanh/mul/reduce of the control einsum run per half (16 chunks) so they
  overlap the second half's chunk stream; the einsum multiply writes tmp in
  b-major order so the reduce is contiguous.
- softplus: DVE |x+b| (clamped at 88) -> ACT exp (psum-resident input) ->
  fused DVE tail (relu(x+b)+u)+u^2(Q1+u*Q2); tmp/mat/u stay f32, so the only
  16-bit roundings are the weights, h1/h2, dr, and the k partials.
"""

import numpy as np
import ml_dtypes
from contextlib import ExitStack

f16 = np.float16

# ---- problem constants (hardcoded per spec) ----
B, T, IN, H, WID, OUT = 64, 64, 32, 128, 128, 1
SUBSTEPS = 2
N_STEPS = (T - 1) * SUBSTEPS  # 126
N_CORES = 8
BL = B // N_CORES  # 8 batch per core
NST = N_STEPS * 6  # 756 stage evals

# Tsit5 tableau
C2, C3, C4, C5 = 0.161, 0.327, 0.9, 0.9800255409045097
A_ROWS = [
    [0.161],
    [-0.008480655492356989, 0.335480655492357],
    [2.8971530571054935, -6.359448489975075, 4.3622954328695815],
    [5.325864828439257, -11.748883564062828, 7.4955393428898365, -0.09249506636175525],
    [5.86145544294642, -12.92096931784711, 8.159367898576159, -0.071584973281401,
     -0.028269050394068383],
]
B_ROW = [0.09646076681806523, 0.01, 0.4798896504144996, 1.379008574103742,
         -3.290069515436081, 2.324710524099774]
C_OFFS = [0.0, C2, C3, C4, C5, 1.0]

# log1p(u) ~= u * (1 + u*(Q1 + u*Q2)) on (0, 1]  (max abs err ~1.3e-3)
Q1 = -0.44593992199872445
Q2 = 0.14039984369167596
A_CLAMP = 88.0  # keeps exp(-a) in the spline's domain; no accuracy effect

# chunk-group split: group A = chunks [0, NA), group B = [NA, 32). Uneven so
# group A's tanh/mul/reduce chain hides under group B's matmul stream.
NA = 26
NB = IN - NA
CA = NA * BL  # psum3-a columns
CB = NB * BL

# family slot layout in wfam: Wa[j][l] = a_{j,l} * W0^T for j=2..6, l=1..j-1
# (15 slots), then Wb[l] = B_l * W0^T for l=1..6 (6 slots).
def _fam_slot(j, l):
    # j = target stage (2..6), l = k index (1..j-1)
    base = sum(range(1, j - 1))  # 0,1,3,6,10 for j=2..6
    return base + (l - 1)


def _famb_slot(l):
    return 15 + (l - 1)


def _fami_slot(l):
    # B_l * Identity — used to accumulate y_{n+1} = y + sum B_l dt k_l in
    # PSUM via zero-stride matmuls straight from the tmp product tiles.
    return 21 + (l - 1)


N_FAM = 27

_CACHE = {}


def _register_custom_ops():
    from concourse.dve_spec import (Spec, Src0, Src1, C0, C1, C2 as C2L, Zero,
                                    relu, sq, maxx, minn)
    from concourse.dve_spec import _has_src1
    from concourse.dve_uop import DveOpSpec
    from concourse.dve_ops import DveOp, OPS, CUSTOM_DVE_SPECS, _SUB_OPCODE_FOR_NAME

    def _make(name, spec):
        if name in _SUB_OPCODE_FOR_NAME:
            for op in OPS:
                if op.name == name:
                    return op
        shas = {}
        for ver in ("v3", "v4"):
            try:
                s = DveOpSpec(name=name, opcode=0, uops=lower_spec(spec, ver=ver),
                              rd1_en=_has_src1(spec))
                shas[ver] = s.sha(ver)
            except Exception:
                pass
        op = DveOp(name, spec, subdim=False, uops_sha=shas)
        OPS.append(op)
        CUSTOM_DVE_SPECS[name] = spec
        _SUB_OPCODE_FOR_NAME[name] = max(_SUB_OPCODE_FOR_NAME.values()) + 1
        assert _SUB_OPCODE_FOR_NAME[name] < 0x20
        return op

    from concourse.dve_spec import lower as lower_spec

    # sp tail: out = (relu(x+b) + u) + u^2*(q1 + u*q2),  u = exp(-|x+b|) from ACT
    def _sp_ref(in0, in1, s0, s1, imm2):
        x = in0.astype(np.float32) + s0
        u = in1.astype(np.float32)
        return (np.maximum(x, 0.0) + u) + (u * u) * (s1 + u * imm2)

    sp_op = _make("SOFTPLUS_TAIL", Spec(
        body=(relu(Src0 + C0) + Src1) + sq(Src1) * (C1 + Src1 * C2L),
        reference=_sp_ref))

    # a = min(|x+b|, clamp)
    def _absc_ref(in0, in1, s0, s1, imm2):
        t = in0.astype(np.float32) + s0
        return np.minimum(np.abs(t), s1)

    t_ = Src0 + C0
    abs_op = _make("ABS_CLAMP", Spec(
        body=minn(maxx(t_, Zero - t_), C1), reference=_absc_ref))

    return sp_op, abs_op


def _build(n_steps):
    import concourse.tile as tile
    import concourse.mybir as mybir
    from concourse import bacc

    f32 = mybir.dt.float32
    fp16 = mybir.dt.float16
    AF = mybir.ActivationFunctionType
    AX = mybir.AxisListType
    ALU = mybir.AluOpType

    SP_OP, ABS_OP = _register_custom_ops()

    nc = bacc.Bacc("TRN2", target_bir_lowering=False, debug=False)

    w0t = nc.declare_dram_parameter("w0t", [H, WID], fp16, isOutput=False)
    w1t = nc.declare_dram_parameter("w1t", [WID, WID], fp16, isOutput=False)
    w2t = nc.declare_dram_parameter("w2t", [WID, H * IN], fp16, isOutput=False)
    wfam = nc.declare_dram_parameter("wfam", [H, N_FAM * WID], fp16, isOutput=False)
    b2iT = nc.declare_dram_parameter("b2iT", [IN, H], fp16, isOutput=False)
    eind = nc.declare_dram_parameter("eind", [IN, IN * BL], fp16, isOutput=False)
    b0c = nc.declare_dram_parameter("b0c", [WID, 1], f32, isOutput=False)
    b1c = nc.declare_dram_parameter("b1c", [WID, 1], f32, isOutput=False)
    bcoef = nc.declare_dram_parameter("bcoef", [128, 12 * BL], f32, isOutput=False)
    dcoef = nc.declare_dram_parameter("dcoef", [NST, BL * IN], fp16, isOutput=False)
    y0t = nc.declare_dram_parameter("y0t", [H, BL], f32, isOutput=False)
    y0bf = nc.declare_dram_parameter("y0bf", [H, BL], fp16, isOutput=False)
    yout = nc.declare_dram_parameter("yout", [H, BL], f32, isOutput=True)

    with tile.TileContext(nc) as tc, ExitStack() as ctx:
        const = ctx.enter_context(tc.tile_pool(name="const", bufs=1))
        hb = ctx.enter_context(tc.tile_pool(name="hb", bufs=3))
        mats = ctx.enter_context(tc.tile_pool(name="mats", bufs=2))
        tmps = ctx.enter_context(tc.tile_pool(name="tmps", bufs=2))
        drp = ctx.enter_context(tc.tile_pool(name="drp", bufs=4))
        pp = ctx.enter_context(tc.tile_pool(name="pp", bufs=2))
        scr = ctx.enter_context(tc.tile_pool(name="scr", bufs=2))
        p1p = ctx.enter_context(tc.tile_pool(name="p1p", bufs=2, space="PSUM"))
        smp = ctx.enter_context(tc.tile_pool(name="smp", bufs=1, space="PSUM"))
        p3ap = ctx.enter_context(tc.tile_pool(name="p3ap", bufs=2, space="PSUM"))
        p3bp = ctx.enter_context(tc.tile_pool(name="p3bp", bufs=2, space="PSUM"))
        yap = ctx.enter_context(tc.tile_pool(name="yap", bufs=1, space="PSUM"))

        w0t_t = const.tile([H, WID], fp16)
        w1t_t = const.tile([WID, WID], fp16)
        w2t_t = const.tile([WID, H * IN], fp16)
        wfam_t = const.tile([H, N_FAM * WID], fp16)
        b2iT_t = const.tile([IN, H], fp16)
        eind_t = const.tile([IN, IN * BL], fp16)
        b0c_t = const.tile([WID, 1], f32)
        b1c_t = const.tile([WID, 1], f32)
        bcoef_t = const.tile([128, 12 * BL], f32)
        y0t_t = const.tile([H, BL], f32)
        y0bf_t = const.tile([H, BL], fp16)
        for t_, d_ in ((w0t_t, w0t), (w1t_t, w1t), (w2t_t, w2t), (wfam_t, wfam),
                       (b2iT_t, b2iT), (eind_t, eind), (b0c_t, b0c), (b1c_t, b1c),
                       (bcoef_t, bcoef), (y0t_t, y0t), (y0bf_t, y0bf)):
            nc.sync.dma_start(t_[:], d_[:, :])

        def fam_ap(slot):
            return wfam_t[:, slot * WID:(slot + 1) * WID]

        # persistent state
        y_st = const.tile([H, BL], f32)
        ybf = const.tile([H, BL], fp16)
        nc.vector.tensor_copy(y_st[:], y0t_t[:])
        nc.vector.tensor_copy(ybf[:], y0bf_t[:])

        def softplus(ps, a_ap, bias_ap, tag):
            # a = |x + b| and u = e^-a back-to-back on ScalarE (same set),
            # then the fused relu/log1p tail on VectorE.
            nc.scalar.activation(a_ap, ps, AF.Abs, bias=bias_ap)
            u = hb.tile([128, BL], f32, tag="u" + tag)
            nc.scalar.activation(u[:], a_ap, AF.Exp, scale=-1.0)
            h = hb.tile([128, BL], fp16, tag="h" + tag)
            nc.vector._custom_dve(SP_OP, out=h[:], in0=ps, in1=u[:],
                                  s0=bias_ap, s1=Q1, imm2=Q2)
            return h

        # bootstrap: psum1 for (n=0, j=1) holds W0 @ y0
        p1_cur = p1p.tile([128, BL], f32, tag="p1")
        nc.tensor.matmul(p1_cur[:], w0t_t[:], y0bf_t[:], start=True, stop=True)

        P_cur = pp.tile([128, 12 * BL], fp16, tag="P")  # dt*k partials, (l, half)
        yacc = yap.tile([128, BL], f32, tag="ya")  # sum_l B_l dt k_l (PSUM)

        for n in range(n_steps):
            for j in range(1, 7):
                s = n * 6 + (j - 1)
                last = (n == n_steps - 1) and (j == 6)

                dr = drp.tile([128, BL * IN], fp16, tag="dr")
                nc.sync.dma_start(
                    dr[:], dcoef[s:s + 1, :].broadcast_to([128, BL * IN]))

                # small-psum tile: p2 = [:,0:8], a1 = [:,8:16], a2 = [:,16:24]
                sm = smp.tile([128, 3 * BL], f32, tag="sm")

                # ---- softplus(psum1) -> h1, mm2, softplus -> h2 ----
                h1 = softplus(p1_cur[:], sm[:, BL:2 * BL], b0c_t[:, 0:1], "1")
                p2 = sm[:, 0:BL]
                nc.tensor.matmul(p2, w1t_t[:], h1[:], start=True, stop=True)
                h2 = softplus(p2, sm[:, 2 * BL:3 * BL], b1c_t[:, 0:1], "2")

                # ---- psum3: two accumulation groups in separate banks
                # (A = chunks [0,NA), B = rest) so group A's tanh can fire
                # while group B's chunks still stream.
                p3a_t = p3ap.tile([128, CA], f32, tag="p3a")
                p3b_t = p3bp.tile([128, CB], f32, tag="p3b")
                p3a = p3a_t[:]
                p3b = p3b_t[:]
                nc.tensor.matmul(p3a, b2iT_t[:], eind_t[:, 0:CA],
                                 start=True, stop=False)
                nc.tensor.matmul(p3b, b2iT_t[:], eind_t[:, CA:],
                                 start=True, stop=False)
                for i in range(IN):
                    if i < NA:
                        out_ap = p3a_t[:, i * BL:(i + 1) * BL]
                        stop = (i == NA - 1)
                    else:
                        out_ap = p3b_t[:, (i - NA) * BL:(i - NA + 1) * BL]
                        stop = (i == IN - 1)
                    nc.tensor.matmul(out_ap, w2t_t[:, i * H:(i + 1) * H],
                                     h2[:], start=False, stop=stop)

                # ---- family pre-accumulation for the NEXT stage's psum1 ----
                if not last:
                    p1_next = p1p.tile([128, BL], f32, tag="p1")
                    nc.tensor.matmul(p1_next[:], w0t_t[:], ybf[:],
                                     start=True, stop=False)
                    if j < 6:
                        slots = [_fam_slot(j + 1, l) for l in range(1, j + 1)]
                    else:
                        slots = [_famb_slot(l) for l in range(1, 7)]
                    # all but the last k are already reduced -> emit now
                    for li, slot in enumerate(slots[:-1]):
                        l = li + 1
                        for hf in range(2):
                            pl = P_cur[:, ((l - 1) * 2 + hf) * BL:
                                       ((l - 1) * 2 + hf + 1) * BL]
                            nc.tensor.matmul(p1_next[:], fam_ap(slot), pl,
                                             start=False, stop=False)

                # ---- per group: tanh -> mul -> direct zero-stride matmuls.
                # The i-summation k = sum_i tmp[:, (i,b)] is linear, so the
                # fresh-partial family matmul and the y-update consume tmp
                # directly: an output AP with stride 0 over i makes PSUM's
                # per-element accumulate logic do the reduction.
                mat_a = mats.tile([128, CA], fp16, tag="mata")
                mat_b = mats.tile([128, CB], fp16, tag="matb")
                tmp_a = tmps.tile([128, CA], fp16, tag="tmpa")
                tmp_b = tmps.tile([128, CB], fp16, tag="tmpb")
                groups = ((p3a, mat_a, tmp_a, CA, NA),
                          (p3b, mat_b, tmp_b, CB, NB))
                for hf, (p3g, matg, tmpg, cg, ng) in enumerate(groups):
                    nc.scalar.activation(matg[:], p3g, AF.Tanh)
                    drg = dr[:, hf * CA:hf * CA + cg]
                    nc.vector.tensor_mul(tmpg[:], matg[:], drg)
                    if not last:
                        nc.tensor.matmul(
                            p1_next[:].unsqueeze(1).broadcast_to([128, ng, BL]),
                            fam_ap(slots[-1]),
                            tmpg[:].rearrange("p (i b) -> p i b", b=BL),
                            start=False, stop=(hf == 1))
                # y-state accumulation: yacc += B_j * sum_i tmp
                for hf, (p3g, matg, tmpg, cg, ng) in enumerate(groups):
                    nc.tensor.matmul(
                        yacc[:].unsqueeze(1).broadcast_to([128, ng, BL]),
                        fam_ap(_fami_slot(j)),
                        tmpg[:].rearrange("p (i b) -> p i b", b=BL),
                        start=(j == 1 and hf == 0), stop=(j == 6 and hf == 1))
                # fp16 dt*k partials (feed the older-l family matmuls only;
                # off the critical path)
                for hf, (p3g, matg, tmpg, cg, ng) in enumerate(groups):
                    pl = P_cur[:, ((j - 1) * 2 + hf) * BL:
                               ((j - 1) * 2 + hf + 1) * BL]
                    with nc.allow_low_precision("fp16 dt*k partials"):
                        nc.vector.tensor_reduce(
                            pl, tmpg[:].rearrange("p (i b) -> p b i", b=BL),
                            axis=AX.X, op=ALU.add)

                # ---- step end: y state update from the yacc psum ----
                if j == 6:
                    nc.vector.tensor_add(y_st[:], y_st[:], yacc[:])
                    nc.vector.tensor_copy(ybf[:], y_st[:])
                    if not last:
                        P_cur = pp.tile([128, 12 * BL], fp16, tag="P")
                        yacc = yap.tile([128, BL], f32, tag="ya")

                if not last:
                    p1_cur = p1_next

        nc.sync.dma_start(yout[:, :], y_st[:])
    nc.compile()
    return nc


def _f32(x):
    return np.float32(x)


def _host_precompute(inputs):
    ts = np.asarray(inputs["ts"], np.float32)
    coeff_d = np.asarray(inputs["coeff_d"], np.float32)
    coeff_c = np.asarray(inputs["coeff_c"], np.float32)
    coeff_b = np.asarray(inputs["coeff_b"], np.float32)
    coeff_a = np.asarray(inputs["coeff_a"], np.float32)
    W0 = np.asarray(inputs["W0"], np.float32)
    W1 = np.asarray(inputs["W1"], np.float32)
    W2 = np.asarray(inputs["W2"], np.float32)
    b0 = np.asarray(inputs["b0"], np.float32)
    b1 = np.asarray(inputs["b1"], np.float32)
    b2 = np.asarray(inputs["b2"], np.float32)

    dt = _f32((ts[-1] - ts[0]) / _f32(N_STEPS))

    # dxdt at all stage times, f32 mirroring the jax reference arithmetic,
    # PRE-SCALED by dt so the on-device partials are dt*k directly.
    d_all = np.empty((NST, B, IN), np.float32)
    for n in range(N_STEPS):
        t0 = _f32(ts[0] + dt * _f32(n))
        for j in range(6):
            tt = _f32(t0 + _f32(C_OFFS[j]) * dt) if j > 0 else t0
            idx = int(np.clip(np.searchsorted(ts, tt, side="right") - 1, 0, T - 2))
            frac = _f32(tt - ts[idx])
            d_all[n * 6 + j] = (coeff_b[:, idx]
                                + frac * (_f32(2.0) * coeff_c[:, idx]
                                          + _f32(3.0) * frac * coeff_d[:, idx]))
    d_all *= dt

    # initial MLP on host (f32, exact as reference)
    x0 = coeff_a[:, 0]
    h = np.maximum(x0 @ np.asarray(inputs["A0"], np.float32).T
                   + np.asarray(inputs["a0"], np.float32), 0)
    h = np.maximum(h @ np.asarray(inputs["A1"], np.float32).T
                   + np.asarray(inputs["a1"], np.float32), 0)
    y0 = (h @ np.asarray(inputs["A2"], np.float32).T
          + np.asarray(inputs["a2"], np.float32)).astype(np.float32)  # [B, H]

    # weights in device layouts (fp16)
    w0t_np = np.ascontiguousarray(W0.T).astype(f16)
    w1t_np = np.ascontiguousarray(W1.T).astype(f16)
    W2r = W2.reshape(H, IN, WID)
    w2t_np = np.ascontiguousarray(
        W2r.transpose(2, 1, 0).reshape(WID, IN * H)).astype(f16)

    wfam_np = np.empty((H, N_FAM * WID), np.float32)
    for j in range(2, 7):
        for l in range(1, j):
            a = _f32(A_ROWS[j - 2][l - 1])
            wfam_np[:, _fam_slot(j, l) * WID:(_fam_slot(j, l) + 1) * WID] = a * W0.T
    for l in range(1, 7):
        bq = _f32(B_ROW[l - 1])
        wfam_np[:, _famb_slot(l) * WID:(_famb_slot(l) + 1) * WID] = bq * W0.T
        wfam_np[:, _fami_slot(l) * WID:(_fami_slot(l) + 1) * WID] = (
            bq * np.eye(H, dtype=np.float32))
    wfam_np = wfam_np.astype(f16)

    b2iT_np = np.ascontiguousarray(b2.reshape(H, IN).T).astype(f16)  # [IN, H]
    eind_np = np.repeat(np.eye(IN, dtype=np.float32), BL, axis=1).astype(f16)

    b0c_np = b0.reshape(WID, 1).copy()
    b1c_np = b1.reshape(WID, 1).copy()

    bcoef_np = np.zeros((128, 12 * BL), np.float32)
    for l in range(6):
        for hf in range(2):
            bcoef_np[:, (l * 2 + hf) * BL:(l * 2 + hf + 1) * BL] = _f32(B_ROW[l])

    per_core = []
    for c in range(N_CORES):
        bs = slice(c * BL, (c + 1) * BL)
        dcoef_np = np.ascontiguousarray(
            d_all[:, bs, :].transpose(0, 2, 1).reshape(NST, IN * BL)).astype(f16)
        y0t_np = np.ascontiguousarray(y0[bs].T)  # [H, BL]
        per_core.append(dict(
            w0t=w0t_np, w1t=w1t_np, w2t=w2t_np, wfam=wfam_np, b2iT=b2iT_np,
            eind=eind_np, b0c=b0c_np, b1c=b1c_np, bcoef=bcoef_np,
            dcoef=dcoef_np, y0t=y0t_np, y0bf=y0t_np.astype(f16)))
    return per_core, y0


def kernel(**inputs):
    from concourse.bass_utils import run_bass_kernel_spmd

    if "nc" not in _CACHE:
        _CACHE["nc"] = _build(N_STEPS)
    nc = _CACHE["nc"]

    in_maps, _ = _host_precompute(inputs)
    res = run_bass_kernel_spmd(nc, in_maps, core_ids=list(range(N_CORES)))
    _CACHE["last_result"] = res

    y = np.empty((B, H), np.float32)
    for c in range(N_CORES):
        y[c * BL:(c + 1) * BL] = res.results[c]["yout"].T

    Wl = np.asarray(inputs["Wl"], np.float32)
    bl = np.asarray(inputs["bl"], np.float32)
    logits = y @ Wl.T + bl
    out = (1.0 / (1.0 + np.exp(-logits)))[:, 0]
    return out.astype(np.float32)
